# revision 38
# baseline (speedup 1.0000x reference)
"""BasicMoEBlock kernel for Trainium2 (Bass/Tile), data-parallel over batch on 8 cores.

Computation per sample (matches the reference):
    rw1 = avgpool_experts(sigmoid(mean_hw(x) @ r1_W.T + r1_b))
    out = relu(bn1(conv3x3(x, rw1 @ e1_w)))
    rw2 = avgpool_experts(sigmoid(mean_hw(out) @ r2_W.T + r2_b))
    out = relu(bn2(conv3x3(out, rw2 @ e2_w)) + x)

Mapping:
  - conv3x3 = 18 accumulating PE matmuls (2 ci-chunks x 9 shifts) over a
    zero-padded 34x34 image held in SBUF (bf16), fp32 PSUM accumulation.
  - per-sample expert-weight combination split across DVE (e0/e1/merges),
    ACT (e2 scale) and GpSimd (e3 fused MAC), bf16.
  - channel pooling for routing rides on ScalarE activation accum_out.
  - routing-weight broadcast across partitions via two tiny PE matmuls.
  - x streams in as bf16 on the Sync HWDGE ring; expert weights stream on
    the ACT HWDGE + GpSimd SWDGE rings in parallel, layer-1 ci0 first.
  - dummy matmuls at t=0 hold the PE HAM clock gate open (2.4 GHz) before
    the first real conv matmul.
  - output is written bf16 and cast back to fp32 on host.
"""

import numpy as np
import ml_dtypes

import concourse.bass as bass
import concourse.tile as tile
from concourse import mybir

F32 = mybir.dt.float32
BF16 = mybir.dt.bfloat16
BF16_NP = ml_dtypes.bfloat16

N_CORES = 8
B_LOC = 4          # samples per core
P = 128            # partitions
CI2 = 2            # channel chunks (256 = 2*128)
C = 256
HW = 1024          # 32*32
PADW = 34
PADHW = PADW * PADW
E = 4              # experts
NSH = 9            # 3x3 shifts
EPS = 1e-5
AF = mybir.ActivationFunctionType
OP = mybir.AluOpType

N_WARM_MM = 16     # dummy matmuls to open the PE clock gate


# ---------------------------------------------------------------- kernel build

def _declare_io(nc):
    d = {}

    def din(name, shape, dtype):
        d[name] = nc.dram_tensor(name, shape, dtype, kind="ExternalInput").ap()

    din("x", [B_LOC, C, HW], BF16)
    din("ew1", [P, E, CI2, NSH * C], BF16)
    din("ew2", [P, E, CI2, NSH * C], BF16)
    din("rwt", [P, 2, CI2, C], BF16)    # r{1,2}_W.T, [cin_in, layer, cin_chunk, interm]
    # fp32 blob: rb1[2] rb2[2] inv1[2] shift1[2] inv2[2] shift2[2] mask4[4]
    din("fblob", [P, 16], F32)
    d["out"] = nc.dram_tensor("out", [B_LOC, C, HW], BF16, kind="ExternalOutput").ap()
    return d


def _emit(tc, d):
    nc = tc.nc

    with (
        tc.tile_pool(name="const", bufs=1) as const,
        tc.tile_pool(name="wcombp", bufs=3) as wcombp,
        tc.tile_pool(name="xin", bufs=3) as xin,
        tc.tile_pool(name="resp", bufs=3) as resp,
        tc.tile_pool(name="rsb", bufs=4) as rsb,
        tc.tile_pool(name="rps", bufs=2, space="PSUM") as rps,
        tc.tile_pool(name="cps", bufs=3, space="PSUM") as cps,
    ):
        # ---- persistent state
        ew_sb = [const.tile([P, E, CI2, NSH * C], BF16, tag=f"ew{l}", name=f"ew{l}") for l in (0, 1)]
        rwt_all = const.tile([P, 2, CI2, C], BF16, tag="rwtall")
        fblob = const.tile([P, 16], F32, tag="fblob")
        rwt_sb = [rwt_all[:, l] for l in (0, 1)]
        pool_bf = [const.tile([P, B_LOC, CI2], BF16, tag=f"poolbf{l}", name=f"poolbf{l}") for l in (0, 1)]
        rb_sb = [fblob[:, 0:2], fblob[:, 2:4]]
        inv_sb = [fblob[:, 4:6], fblob[:, 8:10]]
        shift_sb = [fblob[:, 6:8], fblob[:, 10:12]]
        mask_sb = fblob[:, 12:16]
        ones_p = const.tile([P, 1], BF16, tag="onesp")
        ones_f = const.tile([1, P], BF16, tag="onesf")
        warm_src = const.tile([P, 640], BF16, tag="warmsrc")
        xpad = const.tile([P, B_LOC, CI2, PADHW], BF16, tag="xpad")
        o1pad = const.tile([P, B_LOC, CI2, PADHW], BF16, tag="o1pad")
        pool_acc = [const.tile([P, B_LOC, CI2], F32, tag=f"pool{l}", name=f"pool{l}") for l in (0, 1)]
        rwbc = [const.tile([P, B_LOC, E], F32, tag=f"rwbc{l}", name=f"rwbc{l}") for l in (0, 1)]

        # ---- PE clock-gate warmup: a stream of dummy matmuls (full 128-row
        # lhsT — a 1-partition lhsT does not register as PE activity) keeps
        # the PE busy from ~t0 so the HAM releases the 1.2 GHz throttle
        # (~3.4us of sustained activity) before the first real conv matmul.
        nc.vector.memset(ones_p, 1.0)
        nc.vector.memset(ones_f, 1.0)
        nc.vector.memset(warm_src, 1.0)
        warm_ps = rps.tile([P, 512], F32, tag="rpsA", name="warmps")
        for i in range(N_WARM_MM):
            nc.tensor.matmul(
                warm_ps, warm_src[:, 0:P], warm_src[:, P:640],
                start=True, stop=True,
            )

        # ---- input DMA. Two rings only: Sync HWDGE + GpSimd SWDGE. (The ACT
        # HWDGE ring is unusable here: descriptor instructions share the
        # in-order Activation queue with pooling/sigmoid compute and wedge
        # it for tens of us.) Rings share the 16 DMA engines in global
        # enqueue order, so issue order == arrival order. Priority: routing
        # consts + x0, layer-1 weights in tap-half granularity (so the first
        # weight combination starts before the full ci-half lands), x1-3,
        # then layer-2 weights.
        xf_tiles = {}

        def load_x(b):
            for c in range(CI2):
                xf = xin.tile([P, HW], BF16, tag="xf", name=f"xf{b}{c}")
                nc.sync.dma_start(out=xf, in_=d["x"][b, c * P : (c + 1) * P, :])
                xf_tiles[b, c] = xf

        HALVES = (slice(0, 5 * C), slice(5 * C, NSH * C))  # taps 0-4 / 5-8
        nc.gpsimd.dma_start(out=fblob, in_=d["fblob"])
        nc.gpsimd.dma_start(out=rwt_all[:, 0], in_=d["rwt"][:, 0])
        load_x(0)
        # layer-1 ci0 split across BOTH rings so the first combination's
        # gating pieces (all four experts' taps 0-4) land together ~15.5us:
        # e0/e1 on GpSimd, e2/e3 on Sync right behind x0.
        for sl in HALVES:
            for e in (0, 1):
                nc.gpsimd.dma_start(out=ew_sb[0][:, e, 0, sl], in_=d["ew1"][:, e, 0, sl])
        for e in (2, 3):
            nc.sync.dma_start(out=ew_sb[0][:, e, 0, HALVES[0]], in_=d["ew1"][:, e, 0, HALVES[0]])
        for e in (2, 3):
            nc.sync.dma_start(out=ew_sb[0][:, e, 0, HALVES[1]], in_=d["ew1"][:, e, 0, HALVES[1]])
        # layer-1 ci1, also split: e0/e1 GpSimd, e2/e3 Sync. x1 rides after
        # ci1 — every byte ahead of ci1 delays the first conv's second half;
        # the ACT-ordering gates below keep sample 1's pooling from
        # head-of-line blocking the ACT queue while it waits for x1.
        for e in (0, 1):
            nc.gpsimd.dma_start(out=ew_sb[0][:, e, 1], in_=d["ew1"][:, e, 1])
        for e in (2, 3):
            nc.sync.dma_start(out=ew_sb[0][:, e, 1], in_=d["ew1"][:, e, 1])
        load_x(1)
        load_x(2)
        load_x(3)
        nc.sync.dma_start(out=rwt_all[:, 1], in_=d["rwt"][:, 1])
        # layer-2 weights: ci0 on the GpSimd ring, ci1 on the Sync ring
        for e in range(E):
            nc.gpsimd.dma_start(out=ew_sb[1][:, e, 0], in_=d["ew2"][:, e, 0])
        for e in range(E):
            nc.sync.dma_start(out=ew_sb[1][:, e, 1], in_=d["ew2"][:, e, 1])

        # NOTE: trn2's ACTIVATE instruction has a single sync-wait slot, so
        # every nc.scalar.activation below is arranged to have at most ONE
        # cross-engine producer whose semaphore value is not already covered.
        for b in range(B_LOC):
            v = xpad.rearrange("p b c (r q) -> p b c r q", r=PADW)
            nc.vector.memset(v[:, b, :, 0:PADW:33, :], 0.0)
            nc.vector.memset(v[:, b, :, 1:33, 0:PADW:33], 0.0)
        vo = o1pad.rearrange("p b c (r q) -> p b c r q", r=PADW)
        nc.vector.memset(vo[:, :, :, 0:PADW:33, :], 0.0)
        nc.vector.memset(vo[:, :, :, 1:33, 0:PADW:33], 0.0)

        # The ACT engine executes in order and the tile scheduler's static
        # ordering sometimes hoists a DMA-blocked op (e.g. a pool copy
        # waiting on a late x[b]) ahead of ready weight-combination scales,
        # head-of-line blocking the queue for ~10us. gated_act() chains
        # every ACT op to the previously emitted one (order only, no
        # semaphore), pinning the queue to emission order.
        act_gate = [None]

        def gated_act(**kw):
            inst = nc.scalar.activation(**kw)
            if act_gate[0] is not None:
                tile.add_dep_helper(
                    inst.ins, act_gate[0].ins, sync=False,
                    reason="act queue order",
                )
            act_gate[0] = inst
            return inst

        # warm the ACT function-table with the sigmoid set as the FIRST ACT
        # instruction (source is a memset tile, so no DMA dependency); the
        # chosen set covers Copy/Relu/Sigmoid so no later table switches.
        warm = rsb.tile([P, 1], F32, tag="warm")
        warm_inst = gated_act(
            out=warm, in_=ones_p, func=AF.Sigmoid, scale=1.0
        )

        # ---- x: copy bf16 into padded layout + channel pooling.
        # Mostly on ACT; sample 0 chunk 1 goes on DVE so both of sample 0's
        # copies run in parallel (shortens the startup critical path).
        # Emitted per sample just before its routing (samples 1-3 from the
        # pipeline loop) so a late x[b] DMA can't head-of-line block the
        # in-order ACT queue ahead of sample 0's weight-combination ops.
        def pool_x(b):
            for c in range(CI2):
                dst = xpad[:, b, c].rearrange("p (r q) -> p r q", r=PADW)[:, 1:33, 1:33]
                srcv = xf_tiles[b, c].rearrange("p (r q) -> p r q", r=32)
                if b == 0 and c == 1:
                    nc.vector.tensor_scalar(
                        out=dst, in0=srcv, scalar1=1.0, scalar2=0.0,
                        op0=OP.mult, op1=OP.add,
                        accum_out=pool_acc[0][:, b, c : c + 1],
                    )
                else:
                    ci_ = gated_act(
                        out=dst, in_=srcv, func=AF.Copy, scale=1.0,
                        accum_out=pool_acc[0][:, b, c : c + 1],
                    )
                    if b == 0 and c == 0:
                        # force warm first in the ACT stream so the sigmoid
                        # table set is resident before any ACTIVATE runs
                        # (add_dep_helper(from, to) = "from depends on to")
                        tile.add_dep_helper(
                            ci_.ins, warm_inst.ins, sync=False,
                            reason="act table preload",
                        )

        pool_x(0)

        def routing(b0, n, l):
            """pool_acc[l][:, b0:b0+n] -> rwbc[l][:, b0:b0+n] for n samples."""
            nc.vector.tensor_copy(
                pool_bf[l][:, b0 : b0 + n], pool_acc[l][:, b0 : b0 + n]
            )
            rt_ps = rps.tile([P, CI2, n], F32, tag="rpsA", name="rtps")
            for ic in range(2):
                for cc in range(2):
                    nc.tensor.matmul(
                        rt_ps[:, ic],
                        rwt_sb[l][:, cc, ic * P : (ic + 1) * P],
                        pool_bf[l][:, b0 : b0 + n, cc],
                        start=(cc == 0),
                        stop=(cc == 1),
                    )
            rt2 = rsb.tile([P, CI2, n], F32, tag="rt2", name="rt2")
            for ic in range(2):
                gated_act(
                    out=rt2[:, ic],
                    in_=rt_ps[:, ic],
                    func=AF.Sigmoid,
                    bias=rb_sb[l][:, ic : ic + 1],
                    scale=1.0 / HW,
                )
            # masked[p, bb, e] = rt2[p, e>>1, bb] * mask[p, e] (bf16)
            rt_g = bass.AP(
                tensor=rt2.tensor,
                offset=rt2.offset,
                ap=[rt2.ap[0], [1, n], [n, 2], [0, 2]],
            )
            msk_g = bass.AP(
                tensor=mask_sb.tensor,
                offset=mask_sb.offset,
                ap=[mask_sb.ap[0], [0, n], [2, 2], [1, 2]],
            )
            masked = rsb.tile([P, n, E], BF16, tag="masked", name="masked")
            nc.vector.tensor_mul(
                masked.rearrange("p b (h i) -> p b h i", h=2), rt_g, msk_g
            )
            rw1p_ps = rps.tile([1, n * E], F32, tag="rpsA", name="rw1p")
            nc.tensor.matmul(
                rw1p_ps, ones_p, masked.rearrange("p b e -> p (b e)"),
                start=True, stop=True,
            )
            rw1p_sb = rsb.tile([1, n * E], BF16, tag="rw1p", name="rw1psb")
            nc.vector.tensor_copy(rw1p_sb, rw1p_ps)
            # broadcast back to all partitions: [P, n*E]
            rwbc_ps = rps.tile([P, n * E], F32, tag="rpsA", name="rwbcps")
            nc.tensor.matmul(rwbc_ps, ones_f, rw1p_sb, start=True, stop=True)
            nc.vector.tensor_copy(
                rwbc[l][:, b0 : b0 + n].rearrange("p b e -> p (b e)"), rwbc_ps
            )

        def wcomb_startup_ci(w, b, l, ci):
            """First combination on the critical path: chunked by tap-halves
            per ci-half (matching the DMA piece size) so the conv can start
            on taps 0-4 while taps 5-8 still combine. DVE does e0/e1 +
            merges, ACT scales e2/e3."""
            for sl in HALVES:
                wv = w[:, ci].rearrange("p s q -> p (s q)")[:, sl]
                nc.vector.tensor_scalar(
                    out=wv, in0=ew_sb[l][:, 0, ci, sl],
                    scalar1=rwbc[l][:, b, 0:1], scalar2=None, op0=OP.mult,
                )
                tmp1 = wcombp.tile([P, 5 * C], BF16, tag="wtmp", name="wt1")
                n = sl.stop - sl.start
                nc.vector.tensor_scalar(
                    out=tmp1[:, :n], in0=ew_sb[l][:, 1, ci, sl],
                    scalar1=rwbc[l][:, b, 1:2], scalar2=None, op0=OP.mult,
                )
                nc.vector.tensor_add(wv, wv, tmp1[:, :n])
                for e in (2, 3):
                    tmp = wcombp.tile([P, 5 * C], BF16, tag="wtmp", name="wt2")
                    gated_act(
                        out=tmp[:, :n], in_=ew_sb[l][:, e, ci, sl],
                        func=AF.Copy, scale=rwbc[l][:, b, e : e + 1],
                    )
                    nc.vector.tensor_add(wv, wv, tmp[:, :n])

        def wcomb_mac(b, l):
            """combined per-sample conv weights: sum_e rw[b,e] * ew[e]  (bf16).
            tensor_scalar runs 4x and tensor_tensor 2x, vs 1x for the fused
            scalar_tensor_tensor -- so multiply into tmp, then add. During
            the layer-1 phase ACT is mostly idle, so e2+e3 multiplies ride
            there; in steady state only e3 does."""
            w = wcombp.tile([P, CI2, NSH, C], BF16, tag="wcomb")
            for ci in range(CI2):
                wcomb_mac_ci(w, b, l, ci)
            return w

        def wcomb_mac_ci(w, b, l, ci):
            act_experts = (2, 3) if l == 0 else (3,)
            wv = w[:, ci].rearrange("p s q -> p (s q)")
            nc.vector.tensor_scalar(
                out=wv, in0=ew_sb[l][:, 0, ci],
                scalar1=rwbc[l][:, b, 0:1], scalar2=None, op0=OP.mult,
            )
            for e in (1, 2, 3):
                tmp = wcombp.tile([P, NSH * C], BF16, tag="wtmp", name="wtmp")
                if e in act_experts:
                    gated_act(
                        out=tmp, in_=ew_sb[l][:, e, ci],
                        func=AF.Copy, scale=rwbc[l][:, b, e : e + 1],
                    )
                else:
                    nc.vector.tensor_scalar(
                        out=tmp, in0=ew_sb[l][:, e, ci],
                        scalar1=rwbc[l][:, b, e : e + 1], scalar2=None,
                        op0=OP.mult,
                    )
                nc.vector.tensor_add(wv, wv, tmp)

        def conv(b, w, srcpad, groups=2, ci_outer=False):
            """3x3 same conv: accumulating matmuls per (co, row-group).
            Returns two [P, 1024] fp32 psum tiles (co chunks). groups>1
            finishes each row-group's accumulation before the next, letting
            the epilogue overlap the tail of the conv. ci_outer runs every
            ci0 matmul (both co halves) before any ci1 — used for the first
            conv, whose ci1 weights are still streaming from HBM."""
            psums = [
                cps.tile([P, HW], F32, tag="convps", name=f"convps{co}")
                for co in range(2)
            ]
            gs_of = lambda co: groups if isinstance(groups, int) else groups[co]

            def mm(co, ci, s, g):
                rows = 32 // gs_of(co)
                src34 = srcpad[:, b, ci].rearrange("p (r q) -> p r q", r=PADW)
                ky, kx = divmod(s, 3)
                rhs = src34[:, ky + g * rows : ky + g * rows + rows, kx : kx + 32]
                nc.tensor.matmul(
                    psums[co][:, g * rows * 32 : (g + 1) * rows * 32],
                    w[:, ci, s, co * P : (co + 1) * P],
                    rhs,
                    start=(ci == 0 and s == 0),
                    stop=(ci == 1 and s == NSH - 1),
                )

            if ci_outer:
                for ci in range(2):
                    for s in range(NSH):
                        for co in range(2):
                            for g in range(gs_of(co)):
                                mm(co, ci, s, g)
            else:
                for co in range(2):
                    for g in range(gs_of(co)):
                        for ci in range(2):
                            for s in range(NSH):
                                mm(co, ci, s, g)
            return psums

        def bn1_relu(b, psums):
            for co in range(2):
                dst = o1pad[:, b, co].rearrange("p (r q) -> p r q", r=PADW)[:, 1:33, 1:33]
                gated_act(
                    out=dst,
                    in_=psums[co].rearrange("p (r q) -> p r q", r=32),
                    func=AF.Relu,
                    bias=shift_sb[0][:, co : co + 1],
                    scale=inv_sb[0][:, co : co + 1],
                    accum_out=pool_acc[1][:, b, co : co + 1],
                )

        def bn2_res(b, psums, groups=(1, 1)):
            for co in range(2):
                gs = groups[co]
                rows = 32 // gs
                res = resp.tile([P, HW], F32, tag="res")
                res2 = resp.tile([P, HW], BF16, tag="res2")
                for g in range(gs):
                    sl = slice(g * rows * 32, (g + 1) * rows * 32)
                    resv = res[:, sl].rearrange("p (r q) -> p r q", r=rows)
                    xv = xpad[:, b, co].rearrange("p (r q) -> p r q", r=PADW)[
                        :, 1 + g * rows : 1 + (g + 1) * rows, 1:33]
                    psv = psums[co][:, sl].rearrange("p (r q) -> p r q", r=rows)
                    # res = psum*inv2 + x ; res = max(res + shift2, 0)
                    nc.vector.scalar_tensor_tensor(
                        out=resv, in0=psv, scalar=inv_sb[1][:, co : co + 1], in1=xv,
                        op0=OP.mult, op1=OP.add,
                    )
                    gated_act(
                        out=res2[:, sl], in_=res[:, sl], func=AF.Relu,
                        bias=shift_sb[1][:, co : co + 1], scale=1.0,
                    )
                    nc.sync.dma_start(
                        out=d["out"][b, co * P : (co + 1) * P, sl], in_=res2[:, sl]
                    )

        # ---- main pipeline
        routing(0, 1, 0)
        w1 = [wcombp.tile([P, CI2, NSH, C], BF16, tag="wcomb", name="w1_0")]
        wcomb_startup_ci(w1[0], 0, 0, 0)
        wcomb_startup_ci(w1[0], 0, 0, 1)
        # bridge the PE clock gate from the end of the first warm block
        # (~16us) to the first conv matmul (~24us) so the HAM doesn't
        # re-throttle during the weight-stream wait
        warm_ps2 = rps.tile([P, 512], F32, tag="rpsA", name="warmps2")
        for i in range(30):
            nc.tensor.matmul(
                warm_ps2, warm_src[:, 0:P], warm_src[:, P:640],
                start=True, stop=True,
            )
        for b in (1, 2, 3):
            pool_x(b)
            routing(b, 1, 0)
            w1.append(wcomb_mac(b, 0))
        w2 = {}
        for b in range(B_LOC):
            ps = conv(b, w1[b], xpad, groups=2, ci_outer=(b == 0))
            bn1_relu(b, ps)
            routing(b, 1, 1)
            w2[b] = wcomb_mac(b, 1)
        for b in range(B_LOC):
            last = b == B_LOC - 1
            ps = conv(b, w2[b], o1pad, groups=2)
            bn2_res(b, ps, groups=(2, 2) if last else (1, 1))


_NC_CACHE = {}


def _build_nc():
    if "nc" not in _NC_CACHE:
        import concourse.bacc as bacc

        # Bacc (not raw Bass): its compile() runs split_sync_waits, which
        # legalizes multi-wait instructions for TRN2's 1-wait-per-inst ISA.
        nc = bacc.Bacc("TRN2", target_bir_lowering=False)
        d = _declare_io(nc)
        with tile.TileContext(nc) as tc:
            _emit(tc, d)
        nc.compile()
        _NC_CACHE["nc"] = nc
    return _NC_CACHE["nc"]


# ---------------------------------------------------------------- host prep

def _prep_ew(e_w):
    # [4, 589824] -> [ci_in(128), e, ci_chunk, (ky kx co)]  bf16
    w = np.asarray(e_w, np.float32).reshape(E, C, CI2, P, 3, 3)
    w = w.transpose(3, 0, 2, 4, 5, 1)  # ci_in, e, ci_chunk, ky, kx, co
    return np.ascontiguousarray(w.reshape(P, E, CI2, NSH * C)).astype(BF16_NP)


def _prep_rwt(rW):
    # [interm, cin] -> transpose -> [cin_in(128), cin_chunk, interm]
    t = np.asarray(rW, np.float32).T.reshape(CI2, P, C).transpose(1, 0, 2)
    return np.ascontiguousarray(t).astype(BF16_NP)


def _prep_vec(v):
    return np.ascontiguousarray(np.asarray(v, np.float32).reshape(CI2, P).T)


def _fold_bn(g, b, m, v):
    inv = np.asarray(g, np.float32) / np.sqrt(np.asarray(v, np.float32) + EPS)
    shift = np.asarray(b, np.float32) - np.asarray(m, np.float32) * inv
    return _prep_vec(inv), _prep_vec(shift)


def _mask4():
    m = np.zeros((P, E), np.float32)
    for e in range(E):
        lo = 64 * (e % 2)
        m[lo : lo + 64, e] = 1.0 / 64.0
    return m


def _prep_inputs(inputs):
    inv1, shift1 = _fold_bn(inputs["bn1_gamma"], inputs["bn1_beta"],
                            inputs["bn1_mean"], inputs["bn1_var"])
    inv2, shift2 = _fold_bn(inputs["bn2_gamma"], inputs["bn2_beta"],
                            inputs["bn2_mean"], inputs["bn2_var"])
    fblob = np.concatenate(
        [_prep_vec(inputs["r1_b"]), _prep_vec(inputs["r2_b"]),
         inv1, shift1, inv2, shift2, _mask4()], axis=1
    )
    rwt = np.stack([_prep_rwt(inputs["r1_W"]), _prep_rwt(inputs["r2_W"])], axis=1)
    shared = {
        "ew1": _prep_ew(inputs["e1_w"]),
        "ew2": _prep_ew(inputs["e2_w"]),
        "rwt": np.ascontiguousarray(rwt),
        "fblob": np.ascontiguousarray(fblob),
    }
    x8 = np.ascontiguousarray(
        np.asarray(inputs["x"], np.float32).reshape(N_CORES, B_LOC, C, HW)
    ).astype(BF16_NP)
    return shared, x8


def _run(inputs, trace=False):
    from concourse.bass_utils import run_bass_kernel_spmd

    nc = _build_nc()
    shared, x8 = _prep_inputs(inputs)
    in_maps = [{"x": x8[c], **shared} for c in range(N_CORES)]
    r = run_bass_kernel_spmd(nc, in_maps, list(range(N_CORES)), trace=trace)
    out = np.stack([np.asarray(r.results[c]["out"]) for c in range(N_CORES)])
    return out.reshape(32, C, 32, 32).astype(np.float32), r


def kernel(**inputs):
    out, _ = _run(inputs, trace=False)
    return out


def _install_ntff_shim():
    """The image's antenv package lacks axon_hooks; recreate it and register
    the ctypes NTFF profile hook the way trn_boot would have."""
    import sys
    import types

    if "antenv.axon_hooks" in sys.modules:
        return
    mod = types.ModuleType("antenv.axon_hooks")
    state = {"hook": None}
    mod.set_axon_ntff_profile_hook = lambda h: state.update(hook=h)
    mod.get_axon_ntff_profile_hook = lambda: state["hook"]
    sys.modules["antenv.axon_hooks"] = mod
    import antenv

    antenv.axon_hooks = mod
    try:
        from trn_agent_boot.trn_boot import _ntff_profile_via_ctypes

        mod.set_axon_ntff_profile_hook(
            _ntff_profile_via_ctypes("/opt/axon/libaxon_pjrt.so")
        )
    except Exception as e:  # degrade to no tracing
        print(f"ntff shim failed: {e}")


def run_traced(inputs):
    _install_ntff_shim()
    out, r = _run(inputs, trace=True)
    return out, r


def run_sim(inputs):
    """CoreSim of core 0's shard. Returns [B_LOC, C, 32, 32]."""
    from concourse.bass_interp import CoreSim

    nc = _build_nc()
    shared, x8 = _prep_inputs(inputs)
    sim = CoreSim(nc)
    for k, v in {"x": x8[0], **shared}.items():
        sim.tensor(k)[:] = v
    sim.simulate()
    return np.asarray(sim.tensor("out")).reshape(B_LOC, C, 32, 32).astype(np.float32).copy()


# revision 40
# speedup vs baseline: 1.3568x; 1.3568x over previous
"""BasicMoEBlock kernel for Trainium2 (Bass/Tile), data-parallel over batch on 8 cores.

Computation per sample (matches the reference):
    rw1 = avgpool_experts(sigmoid(mean_hw(x) @ r1_W.T + r1_b))
    out = relu(bn1(conv3x3(x, rw1 @ e1_w)))
    rw2 = avgpool_experts(sigmoid(mean_hw(out) @ r2_W.T + r2_b))
    out = relu(bn2(conv3x3(out, rw2 @ e2_w)) + x)

Mapping:
  - conv3x3 = 18 accumulating PE matmuls (2 ci-chunks x 9 shifts) over a
    zero-padded 34x34 image held in SBUF (bf16), fp32 PSUM accumulation.
  - per-sample expert-weight combination split across DVE (e0/e1/merges),
    ACT (e2 scale) and GpSimd (e3 fused MAC), bf16.
  - channel pooling for routing rides on ScalarE activation accum_out.
  - routing-weight broadcast across partitions via two tiny PE matmuls.
  - x streams in as bf16 on the Sync HWDGE ring; expert weights stream on
    the ACT HWDGE + GpSimd SWDGE rings in parallel, layer-1 ci0 first.
  - dummy matmuls at t=0 hold the PE HAM clock gate open (2.4 GHz) before
    the first real conv matmul.
  - output is written bf16 and cast back to fp32 on host.
"""

import numpy as np
import ml_dtypes

import concourse.bass as bass
import concourse.tile as tile
from concourse import mybir

F32 = mybir.dt.float32
BF16 = mybir.dt.bfloat16
BF16_NP = ml_dtypes.bfloat16

N_CORES = 8
B_LOC = 4          # samples per core
P = 128            # partitions
CI2 = 2            # channel chunks (256 = 2*128)
C = 256
HW = 1024          # 32*32
PADW = 34
PADHW = PADW * PADW
E = 4              # experts
NSH = 9            # 3x3 shifts
EPS = 1e-5
AF = mybir.ActivationFunctionType
OP = mybir.AluOpType

N_WARM_MM = 16     # dummy matmuls to open the PE clock gate


# ---------------------------------------------------------------- kernel build

def _declare_io(nc):
    d = {}

    def din(name, shape, dtype):
        d[name] = nc.dram_tensor(name, shape, dtype, kind="ExternalInput").ap()

    din("x", [B_LOC, C, HW], BF16)
    din("ew1", [P, E, CI2, NSH * C], BF16)
    din("ew2", [P, E, CI2, NSH * C], BF16)
    din("rwt", [P, 2, CI2, C], BF16)    # r{1,2}_W.T, [cin_in, layer, cin_chunk, interm]
    # fp32 blob: rb1[2] rb2[2] inv1[2] shift1[2] inv2[2] shift2[2] mask4[4]
    din("fblob", [P, 16], F32)
    d["out"] = nc.dram_tensor("out", [B_LOC, C, HW], BF16, kind="ExternalOutput").ap()
    return d


def _emit(tc, d):
    nc = tc.nc

    with (
        tc.tile_pool(name="const", bufs=1) as const,
        tc.tile_pool(name="wcombp", bufs=3) as wcombp,
        tc.tile_pool(name="xin", bufs=3) as xin,
        tc.tile_pool(name="resp", bufs=3) as resp,
        tc.tile_pool(name="rsb", bufs=4) as rsb,
        tc.tile_pool(name="rps", bufs=2, space="PSUM") as rps,
        tc.tile_pool(name="cps", bufs=3, space="PSUM") as cps,
    ):
        # ---- persistent state
        ew_sb = [const.tile([P, E, CI2, NSH * C], BF16, tag=f"ew{l}", name=f"ew{l}") for l in (0, 1)]
        rwt_all = const.tile([P, 2, CI2, C], BF16, tag="rwtall")
        fblob = const.tile([P, 16], F32, tag="fblob")
        rwt_sb = [rwt_all[:, l] for l in (0, 1)]
        pool_bf = [const.tile([P, B_LOC, CI2], BF16, tag=f"poolbf{l}", name=f"poolbf{l}") for l in (0, 1)]
        rb_sb = [fblob[:, 0:2], fblob[:, 2:4]]
        inv_sb = [fblob[:, 4:6], fblob[:, 8:10]]
        shift_sb = [fblob[:, 6:8], fblob[:, 10:12]]
        mask_sb = fblob[:, 12:16]
        ones_p = const.tile([P, 1], BF16, tag="onesp")
        ones_f = const.tile([1, P], BF16, tag="onesf")
        warm_src = const.tile([P, 640], BF16, tag="warmsrc")
        xpad = const.tile([P, B_LOC, CI2, PADHW], BF16, tag="xpad")
        o1pad = const.tile([P, B_LOC, CI2, PADHW], BF16, tag="o1pad")
        pool_acc = [const.tile([P, B_LOC, CI2], F32, tag=f"pool{l}", name=f"pool{l}") for l in (0, 1)]
        rwbc = [const.tile([P, B_LOC, E], F32, tag=f"rwbc{l}", name=f"rwbc{l}") for l in (0, 1)]

        # ---- PE clock-gate warmup: a stream of dummy matmuls (full 128-row
        # lhsT — a 1-partition lhsT does not register as PE activity) keeps
        # the PE busy from ~t0 so the HAM releases the 1.2 GHz throttle
        # (~3.4us of sustained activity) before the first real conv matmul.
        nc.vector.memset(ones_p, 1.0)
        nc.vector.memset(ones_f, 1.0)
        nc.vector.memset(warm_src, 1.0)
        warm_ps = rps.tile([P, 512], F32, tag="rpsA", name="warmps")
        for i in range(N_WARM_MM):
            nc.tensor.matmul(
                warm_ps, warm_src[:, 0:P], warm_src[:, P:640],
                start=True, stop=True,
            )

        # ---- input DMA. Two rings only: Sync HWDGE + GpSimd SWDGE. (The ACT
        # HWDGE ring is unusable here: descriptor instructions share the
        # in-order Activation queue with pooling/sigmoid compute and wedge
        # it for tens of us.) Rings share the 16 DMA engines in global
        # enqueue order, so issue order == arrival order. Priority: routing
        # consts + x0, layer-1 weights in tap-half granularity (so the first
        # weight combination starts before the full ci-half lands), x1-3,
        # then layer-2 weights.
        xf_tiles = {}

        def load_x(b):
            for c in range(CI2):
                xf = xin.tile([P, HW], BF16, tag="xf", name=f"xf{b}{c}")
                nc.sync.dma_start(out=xf, in_=d["x"][b, c * P : (c + 1) * P, :])
                xf_tiles[b, c] = xf

        HALVES = (slice(0, 5 * C), slice(5 * C, NSH * C))  # taps 0-4 / 5-8
        nc.gpsimd.dma_start(out=fblob, in_=d["fblob"])
        nc.gpsimd.dma_start(out=rwt_all[:, 0], in_=d["rwt"][:, 0])
        load_x(0)
        # layer-1 ci0 split across BOTH rings so the first combination's
        # gating pieces (all four experts' taps 0-4) land together ~15.5us:
        # e0/e1 on GpSimd, e2/e3 on Sync right behind x0.
        for sl in HALVES:
            for e in (0, 1):
                nc.gpsimd.dma_start(out=ew_sb[0][:, e, 0, sl], in_=d["ew1"][:, e, 0, sl])
        for e in (2, 3):
            nc.sync.dma_start(out=ew_sb[0][:, e, 0, HALVES[0]], in_=d["ew1"][:, e, 0, HALVES[0]])
        load_x(1)
        for e in (2, 3):
            nc.sync.dma_start(out=ew_sb[0][:, e, 0, HALVES[1]], in_=d["ew1"][:, e, 0, HALVES[1]])
        # layer-1 ci1, also split: e0/e1 GpSimd, e2/e3 Sync
        for e in (0, 1):
            nc.gpsimd.dma_start(out=ew_sb[0][:, e, 1], in_=d["ew1"][:, e, 1])
        for e in (2, 3):
            nc.sync.dma_start(out=ew_sb[0][:, e, 1], in_=d["ew1"][:, e, 1])
        load_x(2)
        load_x(3)
        nc.sync.dma_start(out=rwt_all[:, 1], in_=d["rwt"][:, 1])
        # layer-2 weights: ci0 on the GpSimd ring, ci1 on the Sync ring
        for e in range(E):
            nc.gpsimd.dma_start(out=ew_sb[1][:, e, 0], in_=d["ew2"][:, e, 0])
        for e in range(E):
            nc.sync.dma_start(out=ew_sb[1][:, e, 1], in_=d["ew2"][:, e, 1])

        # NOTE: trn2's ACTIVATE instruction has a single sync-wait slot, so
        # every nc.scalar.activation below is arranged to have at most ONE
        # cross-engine producer whose semaphore value is not already covered.
        for b in range(B_LOC):
            v = xpad.rearrange("p b c (r q) -> p b c r q", r=PADW)
            nc.vector.memset(v[:, b, :, 0:PADW:33, :], 0.0)
            nc.vector.memset(v[:, b, :, 1:33, 0:PADW:33], 0.0)
        vo = o1pad.rearrange("p b c (r q) -> p b c r q", r=PADW)
        nc.vector.memset(vo[:, :, :, 0:PADW:33, :], 0.0)
        nc.vector.memset(vo[:, :, :, 1:33, 0:PADW:33], 0.0)

        # warm the ACT function-table with the sigmoid set as the FIRST ACT
        # instruction (source is a memset tile, so no DMA dependency); the
        # chosen set covers Copy/Relu/Sigmoid so no later table switches.
        warm = rsb.tile([P, 1], F32, tag="warm")
        warm_inst = nc.scalar.activation(
            out=warm, in_=ones_p, func=AF.Sigmoid, scale=1.0
        )

        # ---- x: copy bf16 into padded layout + channel pooling.
        # Mostly on ACT; sample 0 chunk 1 goes on DVE so both of sample 0's
        # copies run in parallel (shortens the startup critical path).
        # Emitted per sample just before its routing (samples 1-3 from the
        # pipeline loop) so a late x[b] DMA can't head-of-line block the
        # in-order ACT queue ahead of sample 0's weight-combination ops.
        def pool_x(b):
            for c in range(CI2):
                dst = xpad[:, b, c].rearrange("p (r q) -> p r q", r=PADW)[:, 1:33, 1:33]
                srcv = xf_tiles[b, c].rearrange("p (r q) -> p r q", r=32)
                if b == 0 and c == 1:
                    nc.vector.tensor_scalar(
                        out=dst, in0=srcv, scalar1=1.0, scalar2=0.0,
                        op0=OP.mult, op1=OP.add,
                        accum_out=pool_acc[0][:, b, c : c + 1],
                    )
                else:
                    ci_ = nc.scalar.activation(
                        out=dst, in_=srcv, func=AF.Copy, scale=1.0,
                        accum_out=pool_acc[0][:, b, c : c + 1],
                    )
                    if b == 0 and c == 0:
                        # force warm first in the ACT stream so the sigmoid
                        # table set is resident before any ACTIVATE runs
                        # (add_dep_helper(from, to) = "from depends on to")
                        tile.add_dep_helper(
                            ci_.ins, warm_inst.ins, sync=False,
                            reason="act table preload",
                        )

        pool_x(0)

        def routing(b0, n, l):
            """pool_acc[l][:, b0:b0+n] -> rwbc[l][:, b0:b0+n] for n samples."""
            nc.vector.tensor_copy(
                pool_bf[l][:, b0 : b0 + n], pool_acc[l][:, b0 : b0 + n]
            )
            rt_ps = rps.tile([P, CI2, n], F32, tag="rpsA", name="rtps")
            for ic in range(2):
                for cc in range(2):
                    nc.tensor.matmul(
                        rt_ps[:, ic],
                        rwt_sb[l][:, cc, ic * P : (ic + 1) * P],
                        pool_bf[l][:, b0 : b0 + n, cc],
                        start=(cc == 0),
                        stop=(cc == 1),
                    )
            rt2 = rsb.tile([P, CI2, n], F32, tag="rt2", name="rt2")
            for ic in range(2):
                nc.scalar.activation(
                    out=rt2[:, ic],
                    in_=rt_ps[:, ic],
                    func=AF.Sigmoid,
                    bias=rb_sb[l][:, ic : ic + 1],
                    scale=1.0 / HW,
                )
            # masked[p, bb, e] = rt2[p, e>>1, bb] * mask[p, e] (bf16)
            rt_g = bass.AP(
                tensor=rt2.tensor,
                offset=rt2.offset,
                ap=[rt2.ap[0], [1, n], [n, 2], [0, 2]],
            )
            msk_g = bass.AP(
                tensor=mask_sb.tensor,
                offset=mask_sb.offset,
                ap=[mask_sb.ap[0], [0, n], [2, 2], [1, 2]],
            )
            masked = rsb.tile([P, n, E], BF16, tag="masked", name="masked")
            nc.vector.tensor_mul(
                masked.rearrange("p b (h i) -> p b h i", h=2), rt_g, msk_g
            )
            rw1p_ps = rps.tile([1, n * E], F32, tag="rpsA", name="rw1p")
            nc.tensor.matmul(
                rw1p_ps, ones_p, masked.rearrange("p b e -> p (b e)"),
                start=True, stop=True,
            )
            rw1p_sb = rsb.tile([1, n * E], BF16, tag="rw1p", name="rw1psb")
            nc.vector.tensor_copy(rw1p_sb, rw1p_ps)
            # broadcast back to all partitions: [P, n*E]
            rwbc_ps = rps.tile([P, n * E], F32, tag="rpsA", name="rwbcps")
            nc.tensor.matmul(rwbc_ps, ones_f, rw1p_sb, start=True, stop=True)
            nc.vector.tensor_copy(
                rwbc[l][:, b0 : b0 + n].rearrange("p b e -> p (b e)"), rwbc_ps
            )

        def wcomb_startup_ci(w, b, l, ci):
            """First combination on the critical path: chunked by tap-halves
            per ci-half (matching the DMA piece size) so the conv can start
            on taps 0-4 while taps 5-8 still combine. DVE does e0/e1 +
            merges, ACT scales e2/e3."""
            for sl in HALVES:
                wv = w[:, ci].rearrange("p s q -> p (s q)")[:, sl]
                nc.vector.tensor_scalar(
                    out=wv, in0=ew_sb[l][:, 0, ci, sl],
                    scalar1=rwbc[l][:, b, 0:1], scalar2=None, op0=OP.mult,
                )
                tmp1 = wcombp.tile([P, 5 * C], BF16, tag="wtmp", name="wt1")
                n = sl.stop - sl.start
                nc.vector.tensor_scalar(
                    out=tmp1[:, :n], in0=ew_sb[l][:, 1, ci, sl],
                    scalar1=rwbc[l][:, b, 1:2], scalar2=None, op0=OP.mult,
                )
                nc.vector.tensor_add(wv, wv, tmp1[:, :n])
                for e in (2, 3):
                    tmp = wcombp.tile([P, 5 * C], BF16, tag="wtmp", name="wt2")
                    nc.scalar.activation(
                        out=tmp[:, :n], in_=ew_sb[l][:, e, ci, sl],
                        func=AF.Copy, scale=rwbc[l][:, b, e : e + 1],
                    )
                    nc.vector.tensor_add(wv, wv, tmp[:, :n])

        def wcomb_mac(b, l):
            """combined per-sample conv weights: sum_e rw[b,e] * ew[e]  (bf16).
            tensor_scalar runs 4x and tensor_tensor 2x, vs 1x for the fused
            scalar_tensor_tensor -- so multiply into tmp, then add. During
            the layer-1 phase ACT is mostly idle, so e2+e3 multiplies ride
            there; in steady state only e3 does."""
            w = wcombp.tile([P, CI2, NSH, C], BF16, tag="wcomb")
            for ci in range(CI2):
                wcomb_mac_ci(w, b, l, ci)
            return w

        def wcomb_mac_ci(w, b, l, ci):
            act_experts = (2, 3) if l == 0 else (3,)
            wv = w[:, ci].rearrange("p s q -> p (s q)")
            nc.vector.tensor_scalar(
                out=wv, in0=ew_sb[l][:, 0, ci],
                scalar1=rwbc[l][:, b, 0:1], scalar2=None, op0=OP.mult,
            )
            for e in (1, 2, 3):
                tmp = wcombp.tile([P, NSH * C], BF16, tag="wtmp", name="wtmp")
                if e in act_experts:
                    nc.scalar.activation(
                        out=tmp, in_=ew_sb[l][:, e, ci],
                        func=AF.Copy, scale=rwbc[l][:, b, e : e + 1],
                    )
                else:
                    nc.vector.tensor_scalar(
                        out=tmp, in0=ew_sb[l][:, e, ci],
                        scalar1=rwbc[l][:, b, e : e + 1], scalar2=None,
                        op0=OP.mult,
                    )
                nc.vector.tensor_add(wv, wv, tmp)

        def conv(b, w, srcpad, groups=2, ci_outer=False):
            """3x3 same conv: accumulating matmuls per (co, row-group).
            Returns two [P, 1024] fp32 psum tiles (co chunks). groups>1
            finishes each row-group's accumulation before the next, letting
            the epilogue overlap the tail of the conv. ci_outer runs every
            ci0 matmul (both co halves) before any ci1 — used for the first
            conv, whose ci1 weights are still streaming from HBM."""
            psums = [
                cps.tile([P, HW], F32, tag="convps", name=f"convps{co}")
                for co in range(2)
            ]
            gs_of = lambda co: groups if isinstance(groups, int) else groups[co]

            def mm(co, ci, s, g):
                rows = 32 // gs_of(co)
                src34 = srcpad[:, b, ci].rearrange("p (r q) -> p r q", r=PADW)
                ky, kx = divmod(s, 3)
                rhs = src34[:, ky + g * rows : ky + g * rows + rows, kx : kx + 32]
                nc.tensor.matmul(
                    psums[co][:, g * rows * 32 : (g + 1) * rows * 32],
                    w[:, ci, s, co * P : (co + 1) * P],
                    rhs,
                    start=(ci == 0 and s == 0),
                    stop=(ci == 1 and s == NSH - 1),
                )

            if ci_outer:
                for ci in range(2):
                    for s in range(NSH):
                        for co in range(2):
                            for g in range(gs_of(co)):
                                mm(co, ci, s, g)
            else:
                for co in range(2):
                    for g in range(gs_of(co)):
                        for ci in range(2):
                            for s in range(NSH):
                                mm(co, ci, s, g)
            return psums

        def bn1_relu(b, psums):
            for co in range(2):
                dst = o1pad[:, b, co].rearrange("p (r q) -> p r q", r=PADW)[:, 1:33, 1:33]
                nc.scalar.activation(
                    out=dst,
                    in_=psums[co].rearrange("p (r q) -> p r q", r=32),
                    func=AF.Relu,
                    bias=shift_sb[0][:, co : co + 1],
                    scale=inv_sb[0][:, co : co + 1],
                    accum_out=pool_acc[1][:, b, co : co + 1],
                )

        def bn2_res(b, psums, groups=(1, 1)):
            for co in range(2):
                gs = groups[co]
                rows = 32 // gs
                res = resp.tile([P, HW], F32, tag="res")
                res2 = resp.tile([P, HW], BF16, tag="res2")
                for g in range(gs):
                    sl = slice(g * rows * 32, (g + 1) * rows * 32)
                    resv = res[:, sl].rearrange("p (r q) -> p r q", r=rows)
                    xv = xpad[:, b, co].rearrange("p (r q) -> p r q", r=PADW)[
                        :, 1 + g * rows : 1 + (g + 1) * rows, 1:33]
                    psv = psums[co][:, sl].rearrange("p (r q) -> p r q", r=rows)
                    # res = psum*inv2 + x ; res = max(res + shift2, 0)
                    nc.vector.scalar_tensor_tensor(
                        out=resv, in0=psv, scalar=inv_sb[1][:, co : co + 1], in1=xv,
                        op0=OP.mult, op1=OP.add,
                    )
                    nc.scalar.activation(
                        out=res2[:, sl], in_=res[:, sl], func=AF.Relu,
                        bias=shift_sb[1][:, co : co + 1], scale=1.0,
                    )
                    nc.sync.dma_start(
                        out=d["out"][b, co * P : (co + 1) * P, sl], in_=res2[:, sl]
                    )

        # ---- main pipeline
        routing(0, 1, 0)
        w1 = [wcombp.tile([P, CI2, NSH, C], BF16, tag="wcomb", name="w1_0")]
        wcomb_startup_ci(w1[0], 0, 0, 0)
        wcomb_startup_ci(w1[0], 0, 0, 1)
        # bridge the PE clock gate from the end of the first warm block
        # (~16us) to the first conv matmul (~24us) so the HAM doesn't
        # re-throttle during the weight-stream wait
        warm_ps2 = rps.tile([P, 512], F32, tag="rpsA", name="warmps2")
        for i in range(38):
            nc.tensor.matmul(
                warm_ps2, warm_src[:, 0:P], warm_src[:, P:640],
                start=True, stop=True,
            )
        for b in (1, 2, 3):
            pool_x(b)
            routing(b, 1, 0)
            w1.append(wcomb_mac(b, 0))
        w2 = {}
        for b in range(B_LOC):
            ps = conv(b, w1[b], xpad, groups=2, ci_outer=(b == 0))
            bn1_relu(b, ps)
            routing(b, 1, 1)
            w2[b] = wcomb_mac(b, 1)
        for b in range(B_LOC):
            last = b == B_LOC - 1
            ps = conv(b, w2[b], o1pad, groups=2)
            bn2_res(b, ps, groups=(2, 2) if last else (1, 1))


_NC_CACHE = {}


def _build_nc():
    if "nc" not in _NC_CACHE:
        import concourse.bacc as bacc

        # Bacc (not raw Bass): its compile() runs split_sync_waits, which
        # legalizes multi-wait instructions for TRN2's 1-wait-per-inst ISA.
        nc = bacc.Bacc("TRN2", target_bir_lowering=False)
        d = _declare_io(nc)
        with tile.TileContext(nc) as tc:
            _emit(tc, d)
        nc.compile()
        _NC_CACHE["nc"] = nc
    return _NC_CACHE["nc"]


# ---------------------------------------------------------------- host prep

def _prep_ew(e_w):
    # [4, 589824] -> [ci_in(128), e, ci_chunk, (ky kx co)]  bf16
    w = np.asarray(e_w, np.float32).reshape(E, C, CI2, P, 3, 3)
    w = w.transpose(3, 0, 2, 4, 5, 1)  # ci_in, e, ci_chunk, ky, kx, co
    return np.ascontiguousarray(w.reshape(P, E, CI2, NSH * C)).astype(BF16_NP)


def _prep_rwt(rW):
    # [interm, cin] -> transpose -> [cin_in(128), cin_chunk, interm]
    t = np.asarray(rW, np.float32).T.reshape(CI2, P, C).transpose(1, 0, 2)
    return np.ascontiguousarray(t).astype(BF16_NP)


def _prep_vec(v):
    return np.ascontiguousarray(np.asarray(v, np.float32).reshape(CI2, P).T)


def _fold_bn(g, b, m, v):
    inv = np.asarray(g, np.float32) / np.sqrt(np.asarray(v, np.float32) + EPS)
    shift = np.asarray(b, np.float32) - np.asarray(m, np.float32) * inv
    return _prep_vec(inv), _prep_vec(shift)


def _mask4():
    m = np.zeros((P, E), np.float32)
    for e in range(E):
        lo = 64 * (e % 2)
        m[lo : lo + 64, e] = 1.0 / 64.0
    return m


def _prep_inputs(inputs):
    inv1, shift1 = _fold_bn(inputs["bn1_gamma"], inputs["bn1_beta"],
                            inputs["bn1_mean"], inputs["bn1_var"])
    inv2, shift2 = _fold_bn(inputs["bn2_gamma"], inputs["bn2_beta"],
                            inputs["bn2_mean"], inputs["bn2_var"])
    fblob = np.concatenate(
        [_prep_vec(inputs["r1_b"]), _prep_vec(inputs["r2_b"]),
         inv1, shift1, inv2, shift2, _mask4()], axis=1
    )
    rwt = np.stack([_prep_rwt(inputs["r1_W"]), _prep_rwt(inputs["r2_W"])], axis=1)
    shared = {
        "ew1": _prep_ew(inputs["e1_w"]),
        "ew2": _prep_ew(inputs["e2_w"]),
        "rwt": np.ascontiguousarray(rwt),
        "fblob": np.ascontiguousarray(fblob),
    }
    x8 = np.ascontiguousarray(
        np.asarray(inputs["x"], np.float32).reshape(N_CORES, B_LOC, C, HW)
    ).astype(BF16_NP)
    return shared, x8


def _run(inputs, trace=False):
    from concourse.bass_utils import run_bass_kernel_spmd

    nc = _build_nc()
    shared, x8 = _prep_inputs(inputs)
    in_maps = [{"x": x8[c], **shared} for c in range(N_CORES)]
    r = run_bass_kernel_spmd(nc, in_maps, list(range(N_CORES)), trace=trace)
    out = np.stack([np.asarray(r.results[c]["out"]) for c in range(N_CORES)])
    return out.reshape(32, C, 32, 32).astype(np.float32), r


def kernel(**inputs):
    out, _ = _run(inputs, trace=False)
    return out


def _install_ntff_shim():
    """The image's antenv package lacks axon_hooks; recreate it and register
    the ctypes NTFF profile hook the way trn_boot would have."""
    import sys
    import types

    if "antenv.axon_hooks" in sys.modules:
        return
    mod = types.ModuleType("antenv.axon_hooks")
    state = {"hook": None}
    mod.set_axon_ntff_profile_hook = lambda h: state.update(hook=h)
    mod.get_axon_ntff_profile_hook = lambda: state["hook"]
    sys.modules["antenv.axon_hooks"] = mod
    import antenv

    antenv.axon_hooks = mod
    try:
        from trn_agent_boot.trn_boot import _ntff_profile_via_ctypes

        mod.set_axon_ntff_profile_hook(
            _ntff_profile_via_ctypes("/opt/axon/libaxon_pjrt.so")
        )
    except Exception as e:  # degrade to no tracing
        print(f"ntff shim failed: {e}")


def run_traced(inputs):
    _install_ntff_shim()
    out, r = _run(inputs, trace=True)
    return out, r


def run_sim(inputs):
    """CoreSim of core 0's shard. Returns [B_LOC, C, 32, 32]."""
    from concourse.bass_interp import CoreSim

    nc = _build_nc()
    shared, x8 = _prep_inputs(inputs)
    sim = CoreSim(nc)
    for k, v in {"x": x8[0], **shared}.items():
        sim.tensor(k)[:] = v
    sim.simulate()
    return np.asarray(sim.tensor("out")).reshape(B_LOC, C, 32, 32).astype(np.float32).copy()
